# revision 1
# baseline (speedup 1.0000x reference)
"""AnyPrecisionLinear (4-bit LUT quantized linear) on 8 Trainium2 NeuronCores.

y[b,t,o] = sum_i x[b,t,i] * lut[o, idx(o,i)] + bias[o]
  idx(o,i) = 4-bit code assembled LSB-first from bit-planes qweight[0:4].

Sharding (column-parallel): qweight/lut/bias sharded along out_features into
8 shards of 512; x replicated; each core computes y[:, :, shard] and the host
concatenates along the feature axis.

Per-core kernel: on-device LUT dequant (16-way select tree: 8 ACT affine ops
from bit-plane 0, then 3 in-place predicated folds by planes 1..3 with int16
masks), weights transposed on the PE into a resident bf16 [i, o] layout, and
a bf16 GEMM (N=512 moving operand) over 128-token tiles split into 4 i-chunk
phases so the matmuls start as soon as the first quarter of the weights is
dequantized.  Partial sums staged in SBUF bf16; bias folded in as a K=1
matmul; f32 accumulation and f32 output.  x is fed pre-tiled (pure host-side
layout) and cast f32->bf16 by the SWDGE DMA engines in flight.
"""
import numpy as np
from concourse import bacc, mybir, tile, masks
from concourse.bass_utils import run_bass_kernel_spmd

dt = mybir.dt
F32, BF16, I32, I16 = dt.float32, dt.bfloat16, dt.int32, dt.int16
Act = mybir.ActivationFunctionType
Alu = mybir.AluOpType

N_CORES = 8
B, TT, IN, OF = 4, 2048, 4096, 4096
T = B * TT
O = OF // N_CORES


def _build(T=8192, IN=4096, O=512, ycap=30):
    n_tt, n_it, n_ob = T // 128, IN // 128, O // 128
    words = IN // 32
    CHW0 = min(32, words)
    widths = [CHW0] * (words // CHW0)
    n_ch = len(widths)
    starts = [sum(widths[:c]) for c in range(n_ch)]

    nc = bacc.Bacc("TRN2", target_bir_lowering=False, debug=False)
    x_d = nc.dram_tensor("x", [n_tt, 128, n_it, 128], F32, kind="ExternalInput")
    qw_d = nc.dram_tensor("qw", [4, O, words], I32, kind="ExternalInput")
    lut_d = nc.dram_tensor("lut", [O, 16], F32, kind="ExternalInput")
    bias_d = nc.dram_tensor("bias", [O], F32, kind="ExternalInput")
    mc_d = nc.dram_tensor("mc", [128, 32], I16, kind="ExternalInput")
    out_d = nc.dram_tensor("out", [T, O], F32, kind="ExternalOutput")

    with tile.TileContext(nc) as tc:
        with tc.tile_pool(name="consts", bufs=1) as consts, \
             tc.tile_pool(name="wpool", bufs=1) as wpool, \
             tc.tile_pool(name="qpool", bufs=1) as qpool, \
             tc.tile_pool(name="dq", bufs=2) as dq, \
             tc.tile_pool(name="xph", bufs=8) as xph, \
             tc.tile_pool(name="ysp", bufs=ycap + 4) as ysp, \
             tc.tile_pool(name="yout", bufs=3) as yout, \
             tc.tile_pool(name="psw", bufs=2, space="PSUM") as psw, \
             tc.tile_pool(name="psg", bufs=6, space="PSUM") as psg:

            ident_bf16 = consts.tile([128, 128], BF16)
            masks.make_identity(nc, ident_bf16[:])
            ones_bf = consts.tile([1, 128], BF16)
            nc.vector.memset(ones_bf[:], 1.0)
            mc_t = consts.tile([128, 32], I16)
            nc.sync.dma_start(mc_t[:], mc_d[:])
            bias_f = consts.tile([1, O], F32)
            bias_bf = consts.tile([1, O], BF16)
            nc.sync.dma_start(bias_f[:], bias_d[:].unsqueeze(0))
            nc.vector.tensor_copy(bias_bf[:], bias_f[:])

            WT = wpool.tile([128, n_it, O], BF16)  # [i-part, i-tile, o]

            qw_ts, cks, dks = [], [], []
            for ob in range(n_ob):
                qw_t = qpool.tile([128, 4, words], I32, name=f"qw_t{ob}", tag=f"qw_t{ob}")
                lut_t = qpool.tile([128, 16], F32, name=f"lut_t{ob}", tag=f"lut_t{ob}")
                nc.sync.dma_start(qw_t[:], qw_d[:, ob*128:(ob+1)*128, :].transpose([1, 0, 2]))
                nc.sync.dma_start(lut_t[:], lut_d[ob*128:(ob+1)*128, :])
                ck = qpool.tile([128, 8], F32, name=f"ck{ob}", tag=f"ck{ob}")
                dk = qpool.tile([128, 8], F32, name=f"dk{ob}", tag=f"dk{ob}")
                nc.vector.tensor_copy(ck[:], lut_t[:, 0:16:2])
                nc.vector.tensor_tensor(dk[:], lut_t[:, 1:16:2], lut_t[:, 0:16:2], Alu.subtract)
                qw_ts.append(qw_t); cks.append(ck); dks.append(dk)

            def dequant_chunk(ch, ob):
                qw_t, ck, dk = qw_ts[ob], cks[ob], dks[ob]
                CHW = widths[ch]
                CHI = CHW * 32
                n_cit = CHI // 128
                wsl = slice(starts[ch], starts[ch] + CHW)

                def bq(p):
                    h16 = qw_t[:, p, wsl].bitcast(I16)
                    return h16.rearrange("p (w h) -> p w h", h=2) \
                              .unsqueeze(3).broadcast_to([128, CHW, 2, 16])
                mc_b = mc_t[:].rearrange("p (h s) -> p h s", h=2) \
                           .unsqueeze(1).broadcast_to([128, CHW, 2, 16])
                q4 = lambda tl: tl[:].rearrange("p (w h s) -> p w h s", h=2, s=16)
                t0 = dq.tile([128, CHI], I16, name="t0", tag="t0")
                b0 = dq.tile([128, CHI], BF16, name="b0", tag="b0")
                m1 = dq.tile([128, CHI], I16, name="m1", tag="m1")
                m2 = dq.tile([128, CHI], I16, name="m2", tag="m2")
                m3 = dq.tile([128, CHI], I16, name="m3", tag="m3")
                nc.vector.tensor_tensor(q4(t0), bq(0), mc_b, Alu.bitwise_and)
                nc.vector.tensor_scalar(b0[:], t0[:], 0, None, Alu.not_equal)
                nc.vector.tensor_tensor(q4(m1), bq(1), mc_b, Alu.bitwise_and)
                nc.vector.tensor_tensor(q4(m2), bq(2), mc_b, Alu.bitwise_and)
                nc.vector.tensor_tensor(q4(m3), bq(3), mc_b, Alu.bitwise_and)
                V = dq.tile([128, 8, CHI], BF16, name="V", tag="V")
                for k in range(8):
                    nc.scalar.activation(V[:, k, :], b0[:], Act.Identity,
                                         bias=ck[:, k:k+1], scale=dk[:, k:k+1])
                nc.vector.copy_predicated(
                    V[:, 0:8:2, :], m1[:].unsqueeze(1).broadcast_to([128, 4, CHI]),
                    V[:, 1:8:2, :])
                nc.vector.copy_predicated(
                    V[:, 0:8:4, :], m2[:].unsqueeze(1).broadcast_to([128, 2, CHI]),
                    V[:, 2:8:4, :])
                nc.vector.copy_predicated(V[:, 0, :], m3[:], V[:, 4, :])
                it0 = starts[ch] * 32 // 128
                wt_ps = psw.tile([128, 1024], BF16, name="wt_ps", tag="wt_ps")
                for j in range(n_cit):
                    nc.tensor.transpose(wt_ps[:, j*128:(j+1)*128],
                                        V[:, 0, j*128:(j+1)*128], ident_bf16[:])
                nc.scalar.copy(WT[:, it0:it0+n_cit, ob*128:(ob+1)*128],
                               wt_ps[:, :n_cit*128].rearrange("p (a b) -> p a b", a=n_cit))

            y_sbs = {}

            def emit_phase(c, tt):
                first, last = (c == 0), (c == n_ch - 1)
                n_cit = widths[c] * 32 // 128
                it0 = starts[c] * 32 // 128
                xt = xph.tile([128, n_cit, 128], BF16, name="xt", tag="xt")
                nc.gpsimd.dma_start(xt[:], x_d[tt][:, it0:it0+n_cit, :])  # casts f32->bf16
                y_ps = psg.tile([128, O], F32, name="y_ps", tag="y_ps")
                for j in range(n_cit):
                    it = it0 + j
                    nc.tensor.matmul(y_ps[:], xt[:, j, :], WT[:, it, :],
                                     start=(j == 0), stop=(j == n_cit - 1 and not first))
                if first:
                    nc.tensor.matmul(y_ps[:], ones_bf[:], bias_bf[:],
                                     start=False, stop=True)
                if first and last:
                    y_o = yout.tile([128, O], F32, name="y_o", tag="y_o")
                    nc.scalar.copy(y_o[:], y_ps[:])
                    nc.sync.dma_start(out_d[tt*128:(tt+1)*128, :], y_o[:])
                elif first:
                    y_sb = ysp.tile([128, O], BF16, name="y_sb", tag="y_sb")
                    nc.scalar.copy(y_sb[:], y_ps[:])
                    y_sbs[tt] = y_sb
                elif last:
                    y_o = yout.tile([128, O], F32, name="y_o", tag="y_o")
                    nc.vector.tensor_tensor(y_o[:], y_ps[:], y_sbs.pop(tt)[:], Alu.add)
                    nc.sync.dma_start(out_d[tt*128:(tt+1)*128, :], y_o[:])
                else:
                    y_sb2 = ysp.tile([128, O], BF16, name="y_sb", tag="y_sb")
                    nc.vector.tensor_tensor(y_sb2[:], y_ps[:], y_sbs[tt][:], Alu.add)
                    y_sbs[tt] = y_sb2

            # Greedy interleaved emission: virtual-time model keeps every
            # engine stream free of head-of-line blocking (in-order engines).
            T0, PER_WORD, TILE0 = 13.0, 1.94, 0.2875
            cum = 0.0
            ready_t = []
            for c in range(n_ch):
                cum += widths[c] * PER_WORD
                ready_t.append(T0 + cum)

            def emit_chunk_group(c):
                for ob in range(n_ob):
                    dequant_chunk(c, ob)

            emit_chunk_group(0); emitted_ch = 1
            if n_ch > 1:
                emit_chunk_group(1); emitted_ch = 2
            nxt = [0] * n_ch
            vt = ready_t[0]
            total = n_tt * n_ch
            done = 0
            while done < total:
                if emitted_ch < n_ch and vt >= ready_t[emitted_ch - 1]:
                    emit_chunk_group(emitted_ch); emitted_ch += 1
                cands = []
                for c in range(n_ch):
                    if nxt[c] >= n_tt or ready_t[c] > vt:
                        continue
                    if c > 0 and nxt[c] >= nxt[c - 1]:
                        continue
                    if c == 0 and n_ch > 1 and nxt[0] - nxt[n_ch - 1] >= ycap:
                        continue
                    cands.append(c)
                if not cands:
                    pend = [ready_t[c] for c in range(n_ch) if nxt[c] < n_tt]
                    vt = max(vt + 0.5, min(pend))
                    continue
                c = min(cands, key=lambda c: (nxt[c], -c))
                emit_phase(c, nxt[c])
                nxt[c] += 1
                done += 1
                vt += TILE0 * widths[c] / 8.0 + 0.4

    nc.compile()
    return nc


def _make_mc():
    row = (np.int16(1) << (np.arange(32, dtype=np.int16) % 16)).astype(np.int16)
    return np.broadcast_to(row[None, :], (128, 32)).copy()


def _make_xt(x2):
    # X5[tt, p, a, t] = x2[tt*128+t, a*128+p] — pure layout change
    return np.ascontiguousarray(
        x2.reshape(T // 128, 128, IN // 128, 128).transpose(0, 3, 2, 1))


_nc_cache = None


def kernel(x, qweight, lut, bias, w_bits):
    global _nc_cache
    assert int(w_bits) == 4, f"kernel hardcodes w_bits=4, got {w_bits}"
    x = np.asarray(x, dtype=np.float32)
    qweight = np.asarray(qweight, dtype=np.int32)
    lut = np.asarray(lut, dtype=np.float32)
    bias = np.asarray(bias, dtype=np.float32)
    assert x.shape == (B, TT, IN) and qweight.shape[1:] == (OF, IN // 32)

    x5 = _make_xt(x.reshape(T, IN))
    mc = _make_mc()
    in_maps = []
    for c in range(N_CORES):
        sl = slice(c * O, (c + 1) * O)
        in_maps.append({
            "x": x5,
            "qw": np.ascontiguousarray(qweight[:4, sl, :]),
            "lut": np.ascontiguousarray(lut[sl, :]),
            "bias": np.ascontiguousarray(bias[sl]),
            "mc": mc,
        })

    if _nc_cache is None:
        _nc_cache = _build(T, IN, O)
    res = run_bass_kernel_spmd(_nc_cache, in_maps, core_ids=list(range(N_CORES)))
    y = np.concatenate([res.results[i]["out"] for i in range(N_CORES)], axis=1)
    return np.ascontiguousarray(y.reshape(B, TT, OF).astype(np.float32))



# revision 4
# speedup vs baseline: 1.1655x; 1.1655x over previous
"""AnyPrecisionLinear (4-bit LUT quantized linear) on 8 Trainium2 NeuronCores.

y[b,t,o] = sum_i x[b,t,i] * lut[o, idx(o,i)] + bias[o]
  idx(o,i) = 4-bit code assembled LSB-first from bit-planes qweight[0:4].

Sharding (column-parallel): qweight/lut/bias sharded along out_features into
8 shards of 512; x replicated; each core computes y[:, :, shard] and the host
concatenates along the feature axis.

Per-core kernel v2:
 - x fed as bf16 from the host (halves HBM read traffic vs f32+DMA-cast) in a
   pre-tiled [tile, i-part, i-chunk, token] layout; all DMA on HWDGE queues.
 - On-device LUT dequant (16-way select tree): bit-plane masks on gpsimd,
   8 ACT affine candidates from plane 0, 3 predicated folds by planes 1..3 on
   vector; weights transposed on the PE into a resident bf16 [i, o] layout.
 - Pass-major GEMM schedule: NA staged token tiles run one PSUM pass per
   dequant chunk (partials staged in SBUF bf16, adds on gpsimd) so matmuls
   start as soon as the first quarter of the weights is ready; the remaining
   tiles run fully fused (one 32-matmul f32 PSUM accumulation, no staging).
 - bias pre-broadcast to [128, O] f32 on the host and fused into the
   PSUM-evacuation add (no PE bias matmuls).
"""
import numpy as np
import ml_dtypes
from concourse import bacc, mybir, tile, masks
from concourse.bass_utils import run_bass_kernel_spmd

dt = mybir.dt
F32, BF16, I32, I16 = dt.float32, dt.bfloat16, dt.int32, dt.int16
Act = mybir.ActivationFunctionType
Alu = mybir.AluOpType

N_CORES = 8
B, TT, IN, OF = 4, 2048, 4096, 4096
T = B * TT
O = OF // N_CORES


def _build(T=8192, IN=4096, O=512, NA=40):
    n_tt, n_it = T // 128, IN // 128          # 64 token tiles, 32 i-tiles
    n_ob = O // 128                           # 4 output blocks
    words = IN // 32                          # 128 int32 words per channel
    n_ch, CHW = 4, 32                         # 4 dequant chunks of 32 words
    CHI = CHW * 32                            # 1024 input features per chunk
    n_cit = CHI // 128                        # 8 i-tiles per chunk

    nc = bacc.Bacc("TRN2", target_bir_lowering=False, debug=False)
    x_d = nc.dram_tensor("x", [n_tt, 128, n_it, 128], BF16, kind="ExternalInput")
    qw_d = nc.dram_tensor("qw", [O, 4, words], I32, kind="ExternalInput")
    lut_d = nc.dram_tensor("lut", [O, 16], F32, kind="ExternalInput")
    biasb_d = nc.dram_tensor("biasb", [128, O], F32, kind="ExternalInput")
    mc_d = nc.dram_tensor("mc", [128, 32], I16, kind="ExternalInput")
    out_d = nc.dram_tensor("out", [T, O], F32, kind="ExternalOutput")

    with tile.TileContext(nc) as tc:
        with tc.tile_pool(name="consts", bufs=1) as consts, \
             tc.tile_pool(name="wpool", bufs=1) as wpool, \
             tc.tile_pool(name="qpool", bufs=1) as qpool, \
             tc.tile_pool(name="dq", bufs=2) as dq, \
             tc.tile_pool(name="wd", bufs=6) as wd, \
             tc.tile_pool(name="xph", bufs=8) as xph, \
             tc.tile_pool(name="xfu", bufs=3) as xfu, \
             tc.tile_pool(name="ysp", bufs=NA + 2) as ysp, \
             tc.tile_pool(name="yout", bufs=4) as yout, \
             tc.tile_pool(name="psw", bufs=2, space="PSUM") as psw, \
             tc.tile_pool(name="psg", bufs=6, space="PSUM") as psg:

            ident_bf16 = consts.tile([128, 128], BF16)
            masks.make_identity(nc, ident_bf16[:])
            mc_t = consts.tile([128, 32], I16)
            nc.sync.dma_start(mc_t[:], mc_d[:])
            biasb = consts.tile([128, O], F32)
            nc.sync.dma_start(biasb[:], biasb_d[:])

            WT = wpool.tile([128, n_it, O], BF16)  # [i-part, i-tile, o]

            qw_ts, cks, dks = [], [], []
            for ob in range(n_ob):
                qw_t = qpool.tile([128, 4, words], I32, name=f"qw_t{ob}", tag=f"qw_t{ob}")
                lut_t = qpool.tile([128, 16], F32, name=f"lut_t{ob}", tag=f"lut_t{ob}")
                nc.sync.dma_start(qw_t[:], qw_d[ob*128:(ob+1)*128, :, :])
                nc.sync.dma_start(lut_t[:], lut_d[ob*128:(ob+1)*128, :])
                ck = qpool.tile([128, 8], F32, name=f"ck{ob}", tag=f"ck{ob}")
                dk = qpool.tile([128, 8], F32, name=f"dk{ob}", tag=f"dk{ob}")
                nc.vector.tensor_copy(ck[:], lut_t[:, 0:16:2])
                nc.vector.tensor_tensor(dk[:], lut_t[:, 1:16:2], lut_t[:, 0:16:2], Alu.subtract)
                qw_ts.append(qw_t); cks.append(ck); dks.append(dk)

            wds = {}

            def dq_front(ch, ob):
                """Masks (gpsimd) + candidates (ACT) + folds (DVE) -> Wd."""
                qw_t, ck, dk = qw_ts[ob], cks[ob], dks[ob]
                wsl = slice(ch * CHW, (ch + 1) * CHW)

                def bq(p):
                    h16 = qw_t[:, p, wsl].bitcast(I16)
                    return h16.rearrange("p (w h) -> p w h", h=2) \
                              .unsqueeze(3).broadcast_to([128, CHW, 2, 16])
                mc_b = mc_t[:].rearrange("p (h s) -> p h s", h=2) \
                           .unsqueeze(1).broadcast_to([128, CHW, 2, 16])
                q4 = lambda tl: tl[:].rearrange("p (w h s) -> p w h s", h=2, s=16)
                t0 = dq.tile([128, CHI], I16, name="t0", tag="t0")
                b0 = dq.tile([128, CHI], BF16, name="b0", tag="b0")
                m1 = dq.tile([128, CHI], I16, name="m1", tag="m1")
                m2 = dq.tile([128, CHI], I16, name="m2", tag="m2")
                m3 = dq.tile([128, CHI], I16, name="m3", tag="m3")
                nc.vector.tensor_tensor(q4(t0), bq(0), mc_b, Alu.bitwise_and)
                nc.vector.tensor_scalar(b0[:], t0[:], 0, None, Alu.not_equal)
                nc.vector.tensor_tensor(q4(m1), bq(1), mc_b, Alu.bitwise_and)
                nc.vector.tensor_tensor(q4(m2), bq(2), mc_b, Alu.bitwise_and)
                nc.vector.tensor_tensor(q4(m3), bq(3), mc_b, Alu.bitwise_and)
                V = dq.tile([128, 8, CHI], BF16, name="V", tag="V")
                for k in range(8):
                    nc.scalar.activation(V[:, k, :], b0[:], Act.Identity,
                                         bias=ck[:, k:k+1], scale=dk[:, k:k+1])
                nc.vector.copy_predicated(
                    V[:, 0:8:2, :], m1[:].unsqueeze(1).broadcast_to([128, 4, CHI]),
                    V[:, 1:8:2, :])
                nc.vector.copy_predicated(
                    V[:, 0:8:4, :], m2[:].unsqueeze(1).broadcast_to([128, 2, CHI]),
                    V[:, 2:8:4, :])
                w_t = wd.tile([128, CHI], BF16, name="wd", tag="wd")
                nc.vector.select(w_t[:], m3[:], V[:, 4, :], V[:, 0, :])
                wds[(ch, ob)] = w_t

            def dq_back(ch, ob):
                """PE transposes + WT copy for a dequanted chunk block."""
                w_t = wds.pop((ch, ob))
                it0 = ch * n_cit
                wt_ps = psw.tile([128, n_cit * 128], BF16, name="wt_ps", tag="wt_ps")
                for j in range(n_cit):
                    nc.tensor.transpose(wt_ps[:, j*128:(j+1)*128],
                                        w_t[:, j*128:(j+1)*128], ident_bf16[:])
                nc.scalar.copy(WT[:, it0:it0+n_cit, ob*128:(ob+1)*128],
                               wt_ps[:].rearrange("p (a b) -> p a b", a=n_cit))

            y_sbs = {}

            def emit_pass(ch, tt, ob_split=False):
                first, last = (ch == 0), (ch == n_ch - 1)
                it0 = ch * n_cit
                xt = xph.tile([128, n_cit, 128], BF16, name="xt", tag="xt")
                nc.sync.dma_start(xt[:], x_d[tt][:, it0:it0+n_cit, :])
                y_ps = psg.tile([128, O], F32, name="y_ps", tag="y_ps")
                if ob_split:
                    for ob in range(n_ob):
                        osl = slice(ob * 128, (ob + 1) * 128)
                        for j in range(n_cit):
                            nc.tensor.matmul(y_ps[:, osl], xt[:, j, :],
                                             WT[:, it0 + j, osl],
                                             start=(j == 0), stop=(j == n_cit - 1))
                else:
                    for j in range(n_cit):
                        nc.tensor.matmul(y_ps[:], xt[:, j, :], WT[:, it0 + j, :],
                                         start=(j == 0), stop=(j == n_cit - 1))
                if first:
                    y_sb = ysp.tile([128, O], BF16, name="y_sb", tag="y_sb")
                    nc.scalar.copy(y_sb[:], y_ps[:])
                    y_sbs[tt] = y_sb
                elif not last:
                    y_sb2 = ysp.tile([128, O], BF16, name="y_sb", tag="y_sb")
                    nc.vector.tensor_tensor(y_sb2[:], y_ps[:], y_sbs[tt][:], Alu.add)
                    y_sbs[tt] = y_sb2
                else:
                    y_o = yout.tile([128, O], F32, name="y_o", tag="y_o")
                    nc.vector.tensor_tensor(y_o[:], y_ps[:], y_sbs.pop(tt)[:], Alu.add)
                    nc.gpsimd.tensor_tensor(y_o[:], y_o[:], biasb[:], Alu.add)
                    nc.sync.dma_start(out_d[tt*128:(tt+1)*128, :], y_o[:])

            def emit_fused(tt):
                xt = xfu.tile([128, n_it, 128], BF16, name="xf", tag="xf")
                for q in range(4):
                    qs = slice(q * (n_it // 4), (q + 1) * (n_it // 4))
                    nc.sync.dma_start(xt[:, qs, :], x_d[tt][:, qs, :])
                y_ps = psg.tile([128, O], F32, name="y_ps", tag="y_ps")
                for j in range(n_it):
                    nc.tensor.matmul(y_ps[:], xt[:, j, :], WT[:, j, :],
                                     start=(j == 0), stop=(j == n_it - 1))
                y_o = yout.tile([128, O], F32, name="y_o", tag="y_o")
                nc.vector.tensor_tensor(y_o[:], y_ps[:], biasb[:], Alu.add)
                nc.sync.dma_start(out_d[tt*128:(tt+1)*128, :], y_o[:])

            # ---- schedule ----
            # Chunk 0: fronts for all obs first (keeps the DVE queue free of
            # fold->ACT stalls), then the PE transposes.
            for ob in range(n_ob):
                dq_front(0, ob)
            for ob in range(n_ob):
                dq_back(0, ob)

            # Staged passes over chunks; chunk ch+1 dequant-front interleaved
            # into pass ch, its PE/scalar back-half appended at the pass end.
            step = max(1, NA // n_ob)
            for ch in range(n_ch):
                for k, tt in enumerate(range(NA)):
                    if ch + 1 < n_ch and k % step == 0 and k // step < n_ob:
                        dq_front(ch + 1, k // step)
                    emit_pass(ch, tt, ob_split=(ch == 0 and k < 8))
                if ch + 1 < n_ch:
                    for ob in range(n_ob):
                        dq_back(ch + 1, ob)

            # Fully fused tiles: single f32 PSUM accumulation, no staging.
            for tt in range(NA, n_tt):
                emit_fused(tt)

    nc.compile()
    return nc


def _make_mc():
    row = (np.int16(1) << (np.arange(32, dtype=np.int16) % 16)).astype(np.int16)
    return np.broadcast_to(row[None, :], (128, 32)).copy()


def _make_xt(x2):
    # X5[tt, p, a, t] = x2[tt*128+t, a*128+p] in bf16 — pure layout change
    xb = x2.astype(ml_dtypes.bfloat16)
    return np.ascontiguousarray(
        xb.reshape(T // 128, 128, IN // 128, 128).transpose(0, 3, 2, 1))


def _make_inmaps(x, qweight, lut, bias):
    x5 = _make_xt(np.asarray(x, np.float32).reshape(T, IN))
    mc = _make_mc()
    in_maps = []
    for c in range(N_CORES):
        sl = slice(c * O, (c + 1) * O)
        in_maps.append({
            "x": x5,
            "qw": np.ascontiguousarray(
                np.asarray(qweight, np.int32)[:4, sl, :].transpose(1, 0, 2)),
            "lut": np.ascontiguousarray(np.asarray(lut, np.float32)[sl, :]),
            "biasb": np.broadcast_to(
                np.asarray(bias, np.float32)[sl][None, :], (128, O)).copy(),
            "mc": mc,
        })
    return in_maps


_nc_cache = None


def kernel(x, qweight, lut, bias, w_bits):
    global _nc_cache
    assert int(w_bits) == 4, f"kernel hardcodes w_bits=4, got {w_bits}"
    x = np.asarray(x, dtype=np.float32)
    assert x.shape == (B, TT, IN) and np.asarray(qweight).shape[1:] == (OF, IN // 32)

    in_maps = _make_inmaps(x, qweight, lut, bias)
    if _nc_cache is None:
        _nc_cache = _build(T, IN, O)
    res = run_bass_kernel_spmd(_nc_cache, in_maps, core_ids=list(range(N_CORES)))
    y = np.concatenate([res.results[i]["out"] for i in range(N_CORES)], axis=1)
    return np.ascontiguousarray(y.reshape(B, TT, OF).astype(np.float32))


# revision 8
# speedup vs baseline: 1.1759x; 1.0089x over previous
"""AnyPrecisionLinear (4-bit LUT quantized linear) on 8 Trainium2 NeuronCores.

y[b,t,o] = sum_i x[b,t,i] * lut[o, idx(o,i)] + bias[o]
  idx(o,i) = 4-bit code assembled LSB-first from bit-planes qweight[0:4].

Sharding (column-parallel): qweight/lut/bias sharded along out_features into
8 shards of 512; x replicated; each core computes y[:, :, shard] and the host
concatenates along the feature axis.

Per-core kernel v2:
 - x fed as bf16 from the host (halves HBM read traffic vs f32+DMA-cast) in a
   pre-tiled [tile, i-part, i-chunk, token] layout; all DMA on HWDGE queues.
 - On-device LUT dequant (16-way select tree): bit-plane masks on gpsimd,
   8 ACT affine candidates from plane 0, 3 predicated folds by planes 1..3 on
   vector; weights transposed on the PE into a resident bf16 [i, o] layout.
 - Pass-major GEMM schedule: NA staged token tiles run one PSUM pass per
   dequant chunk (partials staged in SBUF bf16, adds on gpsimd) so matmuls
   start as soon as the first quarter of the weights is ready; the remaining
   tiles run fully fused (one 32-matmul f32 PSUM accumulation, no staging).
 - bias pre-broadcast to [128, O] f32 on the host and fused into the
   PSUM-evacuation add (no PE bias matmuls).
"""
import numpy as np
import ml_dtypes
from concourse import bacc, mybir, tile, masks
from concourse.bass_utils import run_bass_kernel_spmd

dt = mybir.dt
F32, BF16, I32, I16 = dt.float32, dt.bfloat16, dt.int32, dt.int16
Act = mybir.ActivationFunctionType
Alu = mybir.AluOpType

N_CORES = 8
B, TT, IN, OF = 4, 2048, 4096, 4096
T = B * TT
O = OF // N_CORES


def _build(T=8192, IN=4096, O=512, NA=40):
    n_tt, n_it = T // 128, IN // 128          # 64 token tiles, 32 i-tiles
    n_ob = O // 128                           # 4 output blocks
    words = IN // 32                          # 128 int32 words per channel
    n_ch, CHW = 4, 32                         # 4 dequant chunks of 32 words
    CHI = CHW * 32                            # 1024 input features per chunk
    n_cit = CHI // 128                        # 8 i-tiles per chunk

    nc = bacc.Bacc("TRN2", target_bir_lowering=False, debug=False)
    x_d = nc.dram_tensor("x", [n_tt, 128, n_it, 128], BF16, kind="ExternalInput")
    qw_d = nc.dram_tensor("qw", [O, 4, words], I32, kind="ExternalInput")
    lut_d = nc.dram_tensor("lut", [O, 16], F32, kind="ExternalInput")
    biasb_d = nc.dram_tensor("biasb", [128, O], F32, kind="ExternalInput")
    mc_d = nc.dram_tensor("mc", [128, 32], I16, kind="ExternalInput")
    out_d = nc.dram_tensor("out", [T, O], F32, kind="ExternalOutput")

    with tile.TileContext(nc) as tc:
        with tc.tile_pool(name="consts", bufs=1) as consts, \
             tc.tile_pool(name="wpool", bufs=1) as wpool, \
             tc.tile_pool(name="qpool", bufs=1) as qpool, \
             tc.tile_pool(name="dq", bufs=2) as dq, \
             tc.tile_pool(name="wd", bufs=6) as wd, \
             tc.tile_pool(name="xph", bufs=8) as xph, \
             tc.tile_pool(name="xfu", bufs=3) as xfu, \
             tc.tile_pool(name="ysp", bufs=NA + 2) as ysp, \
             tc.tile_pool(name="yout", bufs=4) as yout, \
             tc.tile_pool(name="psw", bufs=2, space="PSUM") as psw, \
             tc.tile_pool(name="psg", bufs=6, space="PSUM") as psg:

            ident_bf16 = consts.tile([128, 128], BF16)
            masks.make_identity(nc, ident_bf16[:])
            mc_t = consts.tile([128, 32], I16)
            biasb = consts.tile([128, O], F32)

            WT = wpool.tile([128, n_it, O], BF16)  # [i-part, i-tile, o]

            # qw plane-0 lands first (it gates the whole dequant chain),
            # then the remaining planes and the small consts.
            qw_ts, cks, dks, lut_ts = [], [], [], []
            for ob in range(n_ob):
                qw_t = qpool.tile([128, 4, words], I32, name=f"qw_t{ob}", tag=f"qw_t{ob}")
                nc.sync.dma_start(qw_t[:, 0, :], qw_d[ob*128:(ob+1)*128, 0, :])
                qw_ts.append(qw_t)
            nc.sync.dma_start(mc_t[:], mc_d[:])
            for ob in range(n_ob):
                lut_t = qpool.tile([128, 16], F32, name=f"lut_t{ob}", tag=f"lut_t{ob}")
                nc.sync.dma_start(lut_t[:], lut_d[ob*128:(ob+1)*128, :])
                lut_ts.append(lut_t)
                nc.sync.dma_start(qw_ts[ob][:, 1:4, :], qw_d[ob*128:(ob+1)*128, 1:4, :])
            for ob in range(n_ob):
                ck = qpool.tile([128, 8], F32, name=f"ck{ob}", tag=f"ck{ob}")
                dk = qpool.tile([128, 8], F32, name=f"dk{ob}", tag=f"dk{ob}")
                nc.vector.tensor_copy(ck[:], lut_ts[ob][:, 0:16:2])
                nc.vector.tensor_tensor(dk[:], lut_ts[ob][:, 1:16:2],
                                        lut_ts[ob][:, 0:16:2], Alu.subtract)
                cks.append(ck); dks.append(dk)
            nc.sync.dma_start(biasb[:], biasb_d[:])

            wds = {}

            def dq_front(ch, ob, direct=False):
                """Masks + candidates (ACT) + folds (DVE) -> Wd (or WT direct)."""
                qw_t, ck, dk = qw_ts[ob], cks[ob], dks[ob]
                wsl = slice(ch * CHW, (ch + 1) * CHW)

                def bq(p):
                    h16 = qw_t[:, p, wsl].bitcast(I16)
                    return h16.rearrange("p (w h) -> p w h", h=2) \
                              .unsqueeze(3).broadcast_to([128, CHW, 2, 16])
                mc_b = mc_t[:].rearrange("p (h s) -> p h s", h=2) \
                           .unsqueeze(1).broadcast_to([128, CHW, 2, 16])
                q4 = lambda tl: tl[:].rearrange("p (w h s) -> p w h s", h=2, s=16)
                t0 = dq.tile([128, CHI], I16, name="t0", tag="t0")
                b0 = dq.tile([128, CHI], BF16, name="b0", tag="b0")
                m1 = dq.tile([128, CHI], I16, name="m1", tag="m1")
                m2 = dq.tile([128, CHI], I16, name="m2", tag="m2")
                m3 = dq.tile([128, CHI], I16, name="m3", tag="m3")
                nc.vector.tensor_tensor(q4(t0), bq(0), mc_b, Alu.bitwise_and)
                nc.vector.tensor_scalar(b0[:], t0[:], 0, None, Alu.not_equal)
                nc.vector.tensor_tensor(q4(m1), bq(1), mc_b, Alu.bitwise_and)
                nc.vector.tensor_tensor(q4(m2), bq(2), mc_b, Alu.bitwise_and)
                nc.vector.tensor_tensor(q4(m3), bq(3), mc_b, Alu.bitwise_and)
                V = dq.tile([128, 8, CHI], BF16, name="V", tag="V")
                for k in range(8):
                    nc.scalar.activation(V[:, k, :], b0[:], Act.Identity,
                                         bias=ck[:, k:k+1], scale=dk[:, k:k+1])
                nc.vector.copy_predicated(
                    V[:, 0:8:2, :], m1[:].unsqueeze(1).broadcast_to([128, 4, CHI]),
                    V[:, 1:8:2, :])
                nc.vector.copy_predicated(
                    V[:, 0:8:4, :], m2[:].unsqueeze(1).broadcast_to([128, 2, CHI]),
                    V[:, 2:8:4, :])
                if direct:
                    # in-place final fold; transposes follow immediately (PE
                    # is idle at startup), so V can die without a Wd copy.
                    nc.vector.copy_predicated(V[:, 0, :], m3[:], V[:, 4, :])
                    _transpose_wt(V[:, 0, :], ch, ob)
                else:
                    w_t = wd.tile([128, CHI], BF16, name="wd", tag="wd")
                    nc.vector.select(w_t[:], m3[:], V[:, 4, :], V[:, 0, :])
                    wds[(ch, ob)] = w_t

            def _transpose_wt(w_ap, ch, ob):
                it0 = ch * n_cit
                wt_ps = psw.tile([128, n_cit * 128], BF16, name="wt_ps", tag="wt_ps")
                for j in range(n_cit):
                    nc.tensor.transpose(wt_ps[:, j*128:(j+1)*128],
                                        w_ap[:, j*128:(j+1)*128], ident_bf16[:])
                nc.scalar.copy(WT[:, it0:it0+n_cit, ob*128:(ob+1)*128],
                               wt_ps[:].rearrange("p (a b) -> p a b", a=n_cit))

            def dq_back(ch, ob):
                """PE transposes + WT copy for a dequanted chunk block."""
                _transpose_wt(wds.pop((ch, ob))[:], ch, ob)

            y_sbs = {}

            def emit_pass(ch, tt, ob_split=False):
                first, last = (ch == 0), (ch == n_ch - 1)
                it0 = ch * n_cit
                xt = xph.tile([128, n_cit, 128], BF16, name="xt", tag="xt")
                nc.sync.dma_start(xt[:], x_d[tt][:, it0:it0+n_cit, :])
                y_ps = psg.tile([128, O], F32, name="y_ps", tag="y_ps")
                if ob_split:
                    for ob in range(n_ob):
                        osl = slice(ob * 128, (ob + 1) * 128)
                        for j in range(n_cit):
                            nc.tensor.matmul(y_ps[:, osl], xt[:, j, :],
                                             WT[:, it0 + j, osl],
                                             start=(j == 0), stop=(j == n_cit - 1))
                else:
                    for j in range(n_cit):
                        nc.tensor.matmul(y_ps[:], xt[:, j, :], WT[:, it0 + j, :],
                                         start=(j == 0), stop=(j == n_cit - 1))
                if first:
                    y_sb = ysp.tile([128, O], BF16, name="y_sb", tag="y_sb")
                    nc.scalar.copy(y_sb[:], y_ps[:])
                    y_sbs[tt] = y_sb
                elif not last:
                    y_sb2 = ysp.tile([128, O], BF16, name="y_sb", tag="y_sb")
                    nc.vector.tensor_tensor(y_sb2[:], y_ps[:], y_sbs[tt][:], Alu.add)
                    y_sbs[tt] = y_sb2
                else:
                    y_o = yout.tile([128, O], F32, name="y_o", tag="y_o")
                    nc.vector.tensor_tensor(y_o[:], y_ps[:], y_sbs.pop(tt)[:], Alu.add)
                    nc.gpsimd.tensor_tensor(y_o[:], y_o[:], biasb[:], Alu.add)
                    nc.sync.dma_start(out_d[tt*128:(tt+1)*128, :], y_o[:])

            def emit_fused(tt, split_out=False):
                xt = xfu.tile([128, n_it, 128], BF16, name="xf", tag="xf")
                for q in range(4):
                    qs = slice(q * (n_it // 4), (q + 1) * (n_it // 4))
                    nc.sync.dma_start(xt[:, qs, :], x_d[tt][:, qs, :])
                y_ps = psg.tile([128, O], F32, name="y_ps", tag="y_ps")
                for j in range(n_it):
                    nc.tensor.matmul(y_ps[:], xt[:, j, :], WT[:, j, :],
                                     start=(j == 0), stop=(j == n_it - 1))
                y_o = yout.tile([128, O], F32, name="y_o", tag="y_o")
                nc.vector.tensor_tensor(y_o[:], y_ps[:], biasb[:], Alu.add)
                if split_out:
                    for h in range(2):
                        osl = slice(h * (O // 2), (h + 1) * (O // 2))
                        nc.sync.dma_start(out_d[tt*128:(tt+1)*128, osl], y_o[:, osl])
                else:
                    nc.sync.dma_start(out_d[tt*128:(tt+1)*128, :], y_o[:])

            # ---- schedule ----
            # Chunk 0: per-ob front + immediate transposes, so the first
            # matmuls can start as soon as ob0 is dequanted.
            for ob in range(n_ob):
                dq_front(0, ob, direct=True)

            # Staged passes over chunks; chunk ch+1 dequant-front interleaved
            # into pass ch, its PE/scalar back-half appended at the pass end.
            step = max(1, NA // n_ob)
            for ch in range(n_ch):
                for k, tt in enumerate(range(NA)):
                    if ch + 1 < n_ch and k % step == 0 and k // step < n_ob:
                        dq_front(ch + 1, k // step)
                    emit_pass(ch, tt, ob_split=(ch == 0 and k < 12))
                if ch + 1 < n_ch:
                    for ob in range(n_ob):
                        dq_back(ch + 1, ob)

            # Fully fused tiles: single f32 PSUM accumulation, no staging.
            for tt in range(NA, n_tt):
                emit_fused(tt, split_out=(tt >= n_tt - 3))

    nc.compile()
    return nc


def _make_mc():
    row = (np.int16(1) << (np.arange(32, dtype=np.int16) % 16)).astype(np.int16)
    return np.broadcast_to(row[None, :], (128, 32)).copy()


def _make_xt(x2):
    # X5[tt, p, a, t] = x2[tt*128+t, a*128+p] in bf16 — pure layout change
    xb = x2.astype(ml_dtypes.bfloat16)
    return np.ascontiguousarray(
        xb.reshape(T // 128, 128, IN // 128, 128).transpose(0, 3, 2, 1))


def _make_inmaps(x, qweight, lut, bias):
    x5 = _make_xt(np.asarray(x, np.float32).reshape(T, IN))
    mc = _make_mc()
    in_maps = []
    for c in range(N_CORES):
        sl = slice(c * O, (c + 1) * O)
        in_maps.append({
            "x": x5,
            "qw": np.ascontiguousarray(
                np.asarray(qweight, np.int32)[:4, sl, :].transpose(1, 0, 2)),
            "lut": np.ascontiguousarray(np.asarray(lut, np.float32)[sl, :]),
            "biasb": np.broadcast_to(
                np.asarray(bias, np.float32)[sl][None, :], (128, O)).copy(),
            "mc": mc,
        })
    return in_maps


_nc_cache = None


def kernel(x, qweight, lut, bias, w_bits):
    global _nc_cache
    assert int(w_bits) == 4, f"kernel hardcodes w_bits=4, got {w_bits}"
    x = np.asarray(x, dtype=np.float32)
    assert x.shape == (B, TT, IN) and np.asarray(qweight).shape[1:] == (OF, IN // 32)

    in_maps = _make_inmaps(x, qweight, lut, bias)
    if _nc_cache is None:
        _nc_cache = _build(T, IN, O)
    res = run_bass_kernel_spmd(_nc_cache, in_maps, core_ids=list(range(N_CORES)))
    y = np.concatenate([res.results[i]["out"] for i in range(N_CORES)], axis=1)
    return np.ascontiguousarray(y.reshape(B, TT, OF).astype(np.float32))
